# revision 5
# baseline (speedup 1.0000x reference)
"""MoE top-2 routing kernel for 8 Trainium2 NeuronCores.

Problem (hardcoded shapes): x [64,8,2048] f32, gate_w [2048,8] f32,
w1/w3 [8,2048,4096] f32, w2 [8,4096,2048] f32, top_k=2.

Strategy (expert parallelism):
  - Host computes the gate (512x8 logits, top-2, softmax) exactly as the
    reference does -- ~17 MFLOP, negligible.
  - Tokens are dispatched per expert (gathered + padded to capacity C),
    one expert per NeuronCore.  Each core runs the SwiGLU FFN for its
    expert over its C token slots:
        outT = w2^T @ (silu(w1^T @ xT) * (w3^T @ xT))
    with all matmuls laid out [K, M]/[K, N] so no on-device transposes
    are needed (tokens are the moving free dim).
  - The combine weights are folded into the host-side scatter-add of the
    per-expert outputs back into the [512, 2048] output.

The kernel is DMA-bound: 50.3 MB of bf16 weights per core at the
~358 GB/s HBM-per-NC limit is ~141 us.  Design choices follow:
  - Weight/x loads alternate between sync (SP HWDGE) and gpsimd
    (SWDGE) so two descriptor streams feed the 16 SDMA rings, and no
    compute instruction ever queues behind a blocking dma_start wait
    (v1 lost ~20 us to Silu stalled behind weight-DMA waits on scalar).
  - Weight dma_starts move 512 KB each ([128 partitions x 4KB lines]);
    8KB lines measured ~12%% slower per byte (packet-split overhead).
  - scalar (Activation): act-table prime, 32 Silus, stage-2 out DMAs
    (issued only after all Silus in program order).
  - vector (DVE): memset, 32 tensor_tensor mults, 16 out copies
    (PSUM f32 -> SBUF bf16).
  - Outputs stream out in [128, 2C] chunks right behind the PSUM
    copies so the final DMA after the last matmul is tiny.
"""

import numpy as np

B, S, D, F, E = 64, 8, 2048, 4096, 8
T = B * S  # 512 tokens
P = 128
KD = D // P   # 16 k-tiles, D contraction
KF = F // P   # 32 k-tiles, F contraction
MF = F // P   # 32 m-tiles, stage 1
MD = D // P   # 16 m-tiles, stage 2
G1 = 4        # stage-1 m-tiles per group (4 gate + 4 up PSUM tiles = 8 banks)
G2 = 8        # stage-2 m-tiles per group (8 PSUM tiles = 8 banks)
NG1 = MF // G1          # 8 stage-1 groups (512 cols each)
NG2 = MD // G2          # 2 stage-2 groups (1024 cols each)
KP1 = KD // 2           # 8 weight DMAs per stage-1 group (512 KB each)
KP2 = KF // 2           # 16 weight DMAs per stage-2 group (512 KB each)
WARMUP = 16

_cache = {}
last_results = None  # BassKernelResults of the most recent device run


def _np_bf16():
    import ml_dtypes
    return np.dtype(ml_dtypes.bfloat16)


def _build(C, w_bufs=36):
    import concourse.mybir as mybir
    import concourse.tile as tile
    from concourse import bacc

    nc = bacc.Bacc(None, target_bir_lowering=False)
    f32 = mybir.dt.float32
    mmdt = mybir.dt.bfloat16

    # weights packed on host so each dma_start moves one [128, 4KB] block:
    #   w13 [NG1, KD//2, 128, kk=2, w=2, G1*128]   (w = gate/up select)
    #   w2p [NG2, KF//2, 128, kk=2, 2, G1*128]     (same 6D block shape)
    w13 = nc.declare_dram_parameter("w13", [NG1, KP1, P, 2, 2, G1 * P],
                                    mmdt, isOutput=False)
    w2p = nc.declare_dram_parameter("w2p", [NG2, KP2, P, 2, 2, G1 * P],
                                    mmdt, isOutput=False)
    xT = nc.declare_dram_parameter("xT", [P, KD, C], mmdt, isOutput=False)
    # out chunks: [g, q, p, mm, c] with d = g*1024 + (q*2+mm)*128 + p
    outT = nc.declare_dram_parameter("outT", [NG2, G2 // 2, P, 2, C], mmdt,
                                     isOutput=True)

    with tile.TileContext(nc) as tc:
        with (
            tc.tile_pool(name="xpool", bufs=1) as xpool,
            tc.tile_pool(name="hpool", bufs=1) as hpool,
            tc.tile_pool(name="wpool", bufs=w_bufs) as wpool,
            tc.tile_pool(name="psum", bufs=8, space="PSUM") as psum,
            tc.tile_pool(name="spool", bufs=4) as spool,
            tc.tile_pool(name="opool", bufs=2) as opool,
        ):
            xt = xpool.tile([P, KD, C], mmdt)
            ht = hpool.tile([P, KF, C], mmdt)

            # Prime the scalar act table (2.6us of ACT_TABLE_LOAD) during
            # warmup instead of stalling the first real Silu.
            warm = xpool.tile([P, 256], mmdt, name="warm")
            nc.vector.memset(warm[:], 0.0)
            prime = xpool.tile([P, 8], f32, name="prime")
            nc.scalar.activation(prime[:], warm[:, 0:8],
                                 mybir.ActivationFunctionType.Silu)

            # First weight tile, then x (the first matmul needs both);
            # remaining weight issues follow in consumption order.
            dma_eng = [nc.sync, nc.gpsimd]
            ndma = 0
            wt0 = wpool.tile([P, 2, 2, G1 * P], mmdt, tag="w", name="wt_0_0")
            nc.sync.dma_start(out=wt0[:], in_=w13[0, 0])
            ndma = 1
            nc.gpsimd.dma_start(out=xt[:, 0:4, :], in_=xT[:, 0:4, :])
            nc.sync.dma_start(out=xt[:, 4:, :], in_=xT[:, 4:, :])

            # HAM warmup: PE activity covering the cold-clock window while
            # the first weight/x transfers land.
            ps_w = psum.tile([P, C], f32, tag="ps", name="ps_warm")
            for i in range(WARMUP):
                nc.tensor.matmul(ps_w[:], warm[:, :P], warm[:, :C],
                                 start=True, stop=True)

            # stage 1: hT[f, t] = silu(w1^T xT) * (w3^T xT), F-major groups
            for g in range(NG1):
                ps_g = [psum.tile([P, C], f32, tag="ps", name=f"ps_g{g}_{m}")
                        for m in range(G1)]
                ps_u = [psum.tile([P, C], f32, tag="ps", name=f"ps_u{g}_{m}")
                        for m in range(G1)]
                for kp in range(KP1):
                    if g == 0 and kp == 0:
                        wt = wt0
                    else:
                        wt = wpool.tile([P, 2, 2, G1 * P], mmdt, tag="w")
                        dma_eng[ndma % 2].dma_start(out=wt[:], in_=w13[g, kp])
                        ndma += 1
                    for kk in range(2):
                        k = kp * 2 + kk
                        st, sp = (k == 0), (k == KD - 1)
                        for m in range(G1):
                            nc.tensor.matmul(ps_g[m][:], wt[:, kk, 0, m * P:(m + 1) * P],
                                             xt[:, k, :], start=st, stop=sp)
                            nc.tensor.matmul(ps_u[m][:], wt[:, kk, 1, m * P:(m + 1) * P],
                                             xt[:, k, :], start=st, stop=sp)
                for m in range(G1):
                    sig = spool.tile([P, C], f32, tag="sig")
                    nc.scalar.activation(sig[:], ps_g[m][:],
                                         mybir.ActivationFunctionType.Silu)
                    nc.vector.tensor_tensor(out=ht[:, g * G1 + m, :], in0=sig[:],
                                            in1=ps_u[m][:], op=mybir.AluOpType.mult)

            # stage 2: outT[d, t] = w2^T @ hT
            for g in range(NG2):
                ps_o = [psum.tile([P, C], f32, tag="ps", name=f"ps_o{g}_{m}")
                        for m in range(G2)]
                for kp in range(KP2):
                    wt = wpool.tile([P, 2, 2, G1 * P], mmdt, tag="w")
                    dma_eng[ndma % 2].dma_start(out=wt[:], in_=w2p[g, kp])
                    ndma += 1
                    for kk in range(2):
                        k = kp * 2 + kk
                        st, sp = (k == 0), (k == KF - 1)
                        for m in range(G2):
                            nc.tensor.matmul(ps_o[m][:],
                                             wt[:, kk, m // 4, (m % 4) * P:(m % 4 + 1) * P],
                                             ht[:, k, :], start=st, stop=sp)
                obuf = opool.tile([P, G2, C], mmdt, tag="o", name=f"ob{g}")
                for m in range(G2):
                    nc.vector.tensor_copy(out=obuf[:, m, :], in_=ps_o[m][:])
                    if m % 2 == 1:
                        nc.scalar.dma_start(out=outT[g, m // 2],
                                            in_=obuf[:, m - 1:m + 1, :])

    nc.compile()
    return nc


def _route(x2d, gate_w, top_k):
    """Replicates the reference gate on host: returns (sel [T,k], cw [T,k])."""
    logits = x2d @ gate_w                       # [T, E] fp32
    sel = np.argsort(-logits, axis=-1, kind="stable")[:, :top_k]
    vals = np.take_along_axis(logits, sel, axis=-1)
    m = vals.max(axis=-1, keepdims=True)
    ex = np.exp(vals - m)
    cw = ex / ex.sum(axis=-1, keepdims=True)
    return sel, cw


def kernel(x, gate_w, w1, w3, w2, top_k):
    from concourse.bass_utils import run_bass_kernel_spmd

    x = np.asarray(x, np.float32)
    gate_w = np.asarray(gate_w, np.float32)
    w1 = np.asarray(w1, np.float32)
    w3 = np.asarray(w3, np.float32)
    w2 = np.asarray(w2, np.float32)
    k = int(top_k)

    x2d = x.reshape(T, D)
    sel, cw = _route(x2d, gate_w, k)

    # token lists per expert
    idx = [np.where((sel == e).any(axis=1))[0] for e in range(E)]
    wgt = []
    for e in range(E):
        m = sel[idx[e]] == e
        wgt.append(cw[idx[e]][m].astype(np.float32))
    counts = np.array([len(i) for i in idx])
    maxc = int(counts.max())
    C = max(160, -(-maxc // 32) * 32)
    n_chunks = 1
    if C > 512:  # capacity overflow: run multiple passes of 512
        C = 512
        n_chunks = -(-maxc // C)

    if C not in _cache:
        _cache[C] = _build(C, w_bufs=36 if C <= 256 else 20)
    nc = _cache[C]

    ndt = _np_bf16()
    wpacked = []
    for e in range(E):
        # w13 [NG1, KD//2, P, kk, w, G1*P]: line = one 4KB block per partition
        w1r = w1[e].astype(ndt).reshape(KP1, 2, P, NG1, G1 * P)
        w3r = w3[e].astype(ndt).reshape(KP1, 2, P, NG1, G1 * P)
        w13 = np.ascontiguousarray(
            np.stack([w1r, w3r], axis=4).transpose(3, 0, 2, 1, 4, 5))
        # w2p [NG2, KF//2, P, kk, 2, G1*P] (same 6D block shape as w13)
        w2r = w2[e].astype(ndt).reshape(KP2, 2, P, NG2, G2 * P)
        w2pk = np.ascontiguousarray(w2r.transpose(3, 0, 2, 1, 4)).reshape(
            NG2, KP2, P, 2, 2, G1 * P)
        wpacked.append((w13, w2pk))

    out = np.zeros((T, D), np.float32)
    for chunk in range(n_chunks):
        in_maps = []
        for e in range(E):
            ide = idx[e][chunk * C:(chunk + 1) * C]
            xTe = np.zeros((D, C), ndt)
            xTe[:, :len(ide)] = x2d[ide].T.astype(ndt)
            in_maps.append({
                "xT": np.ascontiguousarray(
                    xTe.reshape(KD, P, C).transpose(1, 0, 2)),
                "w13": wpacked[e][0],
                "w2p": wpacked[e][1],
            })
        res = run_bass_kernel_spmd(nc, in_maps, core_ids=list(range(E)))
        global last_results
        last_results = res
        for e in range(E):
            ide = idx[e][chunk * C:(chunk + 1) * C]
            if len(ide) == 0:
                continue
            we = wgt[e][chunk * C:(chunk + 1) * C]
            # outT [NG2, 4, P, 2, C] -> [D, C], d = g*1024 + (q*2+mm)*128 + p
            oTe = res.results[e]["outT"].astype(np.float32).transpose(
                0, 1, 3, 2, 4).reshape(D, C)
            # token indices are unique within one expert's list
            out[ide] += we[:, None] * oTe[:, :len(ide)].T

    return out.reshape(B, S, D)


# revision 6
# speedup vs baseline: 1.0357x; 1.0357x over previous
"""MoE top-2 routing kernel for 8 Trainium2 NeuronCores.

Problem (hardcoded shapes): x [64,8,2048] f32, gate_w [2048,8] f32,
w1/w3 [8,2048,4096] f32, w2 [8,4096,2048] f32, top_k=2.

Strategy (expert parallelism):
  - Host computes the gate (512x8 logits, top-2, softmax) exactly as the
    reference does -- ~17 MFLOP, negligible.
  - Tokens are dispatched per expert (gathered + padded to capacity C),
    one expert per NeuronCore.  Each core runs the SwiGLU FFN for its
    expert over its C token slots:
        outT = w2^T @ (silu(w1^T @ xT) * (w3^T @ xT))
    with all matmuls laid out [K, M]/[K, N] so no on-device transposes
    are needed (tokens are the moving free dim).
  - The combine weights are folded into the host-side scatter-add of the
    per-expert outputs back into the [512, 2048] output.

The kernel is DMA-bound: 50.3 MB of bf16 weights per core at the
~358 GB/s HBM-per-NC limit is ~141 us.  Design choices follow:
  - Weight/x loads alternate between sync (SP HWDGE) and gpsimd
    (SWDGE) so two descriptor streams feed the 16 SDMA rings, and no
    compute instruction ever queues behind a blocking dma_start wait
    (v1 lost ~20 us to Silu stalled behind weight-DMA waits on scalar).
  - Weight dma_starts move 512 KB each ([128 partitions x 4KB lines]);
    8KB lines measured ~12%% slower per byte (packet-split overhead).
  - scalar (Activation): act-table prime, 32 Silus, stage-2 out DMAs
    (issued only after all Silus in program order).
  - vector (DVE): memset, 32 tensor_tensor mults, 16 out copies
    (PSUM f32 -> SBUF bf16).
  - Outputs stream out in [128, 2C] chunks right behind the PSUM
    copies so the final DMA after the last matmul is tiny.
"""

import numpy as np

B, S, D, F, E = 64, 8, 2048, 4096, 8
T = B * S  # 512 tokens
P = 128
KD = D // P   # 16 k-tiles, D contraction
KF = F // P   # 32 k-tiles, F contraction
MF = F // P   # 32 m-tiles, stage 1
MD = D // P   # 16 m-tiles, stage 2
G1 = 4        # stage-1 m-tiles per group (4 gate + 4 up PSUM tiles = 8 banks)
G2 = 8        # stage-2 m-tiles per group (8 PSUM tiles = 8 banks)
NG1 = MF // G1          # 8 stage-1 groups (512 cols each)
NG2 = MD // G2          # 2 stage-2 groups (1024 cols each)
KP1 = KD // 2           # 8 weight DMAs per stage-1 group (512 KB each)
KP2 = KF // 2           # 16 weight DMAs per stage-2 group (512 KB each)
WARMUP = 16

_cache = {}
last_results = None  # BassKernelResults of the most recent device run


def _np_bf16():
    import ml_dtypes
    return np.dtype(ml_dtypes.bfloat16)


def _build(C, w_bufs=24):
    import concourse.mybir as mybir
    import concourse.tile as tile
    from concourse import bacc

    nc = bacc.Bacc(None, target_bir_lowering=False)
    f32 = mybir.dt.float32
    mmdt = mybir.dt.bfloat16

    # weights packed on host so each dma_start moves one [128, 4KB] block:
    #   w13 [NG1, KD//2, 128, kk=2, w=2, G1*128]   (w = gate/up select)
    #   w2p [NG2, KF//2, 128, kk=2, 2, G1*128]     (same 6D block shape)
    w13 = nc.declare_dram_parameter("w13", [NG1, KP1, P, 2, 2, G1 * P],
                                    mmdt, isOutput=False)
    w2p = nc.declare_dram_parameter("w2p", [NG2, KP2, P, 2, 2, G1 * P],
                                    mmdt, isOutput=False)
    xT = nc.declare_dram_parameter("xT", [P, KD, C], mmdt, isOutput=False)
    # out chunks: [g, q, p, mm, c] with d = g*1024 + (q*2+mm)*128 + p
    outT = nc.declare_dram_parameter("outT", [NG2, G2 // 2, P, 2, C], mmdt,
                                     isOutput=True)

    with tile.TileContext(nc) as tc:
        with (
            tc.tile_pool(name="xpool", bufs=1) as xpool,
            tc.tile_pool(name="hpool", bufs=1) as hpool,
            tc.tile_pool(name="wpool", bufs=w_bufs) as wpool,
            tc.tile_pool(name="wpoolB", bufs=12) as wpoolB,
            tc.tile_pool(name="psum", bufs=8, space="PSUM") as psum,
            tc.tile_pool(name="spool", bufs=4) as spool,
            tc.tile_pool(name="opool", bufs=2) as opool,
        ):
            xt = xpool.tile([P, KD, C], mmdt)
            ht = hpool.tile([P, KF, C], mmdt)

            # Prime the scalar act table (2.6us of ACT_TABLE_LOAD) during
            # warmup instead of stalling the first real Silu.
            warm = xpool.tile([P, 256], mmdt, name="warm")
            nc.vector.memset(warm[:], 0.0)
            prime = xpool.tile([P, 8], f32, name="prime")
            nc.scalar.activation(prime[:], warm[:, 0:8],
                                 mybir.ActivationFunctionType.Silu)

            # Scalar-issued weight tiles come from a dedicated small pool
            # (wpoolB) and are issued >= 1 group ahead of consumption, so
            # their buffer-free waits are statically satisfied and the
            # Silus behind them in the scalar FIFO never stall (the v1
            # priority inversion).  Sync-issued tiles carry the
            # backpressure on the deep wpool.
            sB = {}

            def issue_s1_scalar(g):
                for kp in range(1, KP1, 2):
                    t = wpoolB.tile([P, 2, 2, G1 * P], mmdt, tag="wB",
                                    name=f"wB1_{g}_{kp}")
                    nc.scalar.dma_start(out=t[:], in_=w13[g, kp])
                    sB[("s1", g, kp)] = t

            def issue_s2_scalar(g, kps):
                for kp in kps:
                    t = wpoolB.tile([P, 2, 2, G1 * P], mmdt, tag="wB",
                                    name=f"wB2_{g}_{kp}")
                    nc.scalar.dma_start(out=t[:], in_=w2p[g, kp])
                    sB[("s2", g, kp)] = t

            # First weight tile + x on sync; scalar pre-issues groups 0/1.
            wt0 = wpool.tile([P, 2, 2, G1 * P], mmdt, tag="w", name="wt_0_0")
            nc.sync.dma_start(out=wt0[:], in_=w13[0, 0])
            issue_s1_scalar(0)
            nc.sync.dma_start(out=xt[:, 0:4, :], in_=xT[:, 0:4, :])
            nc.sync.dma_start(out=xt[:, 4:, :], in_=xT[:, 4:, :])
            issue_s1_scalar(1)

            # HAM warmup: PE activity covering the cold-clock window while
            # the first weight/x transfers land.
            ps_w = psum.tile([P, C], f32, tag="ps", name="ps_warm")
            for i in range(WARMUP):
                nc.tensor.matmul(ps_w[:], warm[:, :P], warm[:, :C],
                                 start=True, stop=True)

            # stage 1: hT[f, t] = silu(w1^T xT) * (w3^T xT), F-major groups
            for g in range(NG1):
                ps_g = [psum.tile([P, C], f32, tag="ps", name=f"ps_g{g}_{m}")
                        for m in range(G1)]
                ps_u = [psum.tile([P, C], f32, tag="ps", name=f"ps_u{g}_{m}")
                        for m in range(G1)]
                for kp in range(KP1):
                    if kp % 2 == 1:
                        wt = sB.pop(("s1", g, kp))
                    elif g == 0 and kp == 0:
                        wt = wt0
                    else:
                        wt = wpool.tile([P, 2, 2, G1 * P], mmdt, tag="w")
                        nc.sync.dma_start(out=wt[:], in_=w13[g, kp])
                    for kk in range(2):
                        k = kp * 2 + kk
                        st, sp = (k == 0), (k == KD - 1)
                        for m in range(G1):
                            nc.tensor.matmul(ps_g[m][:], wt[:, kk, 0, m * P:(m + 1) * P],
                                             xt[:, k, :], start=st, stop=sp)
                            nc.tensor.matmul(ps_u[m][:], wt[:, kk, 1, m * P:(m + 1) * P],
                                             xt[:, k, :], start=st, stop=sp)
                for m in range(G1):
                    sig = spool.tile([P, C], f32, tag="sig")
                    nc.scalar.activation(sig[:], ps_g[m][:],
                                         mybir.ActivationFunctionType.Silu)
                    nc.vector.tensor_tensor(out=ht[:, g * G1 + m, :], in0=sig[:],
                                            in1=ps_u[m][:], op=mybir.AluOpType.mult)
                if g + 2 < NG1:
                    issue_s1_scalar(g + 2)
                elif g == NG1 - 2:
                    issue_s2_scalar(0, range(1, KP2 // 2, 2))
                else:  # g == NG1 - 1
                    issue_s2_scalar(0, range(KP2 // 2 + 1, KP2, 2))

            # stage 2: outT[d, t] = w2^T @ hT
            for g in range(NG2):
                ps_o = [psum.tile([P, C], f32, tag="ps", name=f"ps_o{g}_{m}")
                        for m in range(G2)]
                for kp in range(KP2):
                    if kp % 2 == 1:
                        wt = sB.pop(("s2", g, kp))
                    else:
                        wt = wpool.tile([P, 2, 2, G1 * P], mmdt, tag="w")
                        nc.sync.dma_start(out=wt[:], in_=w2p[g, kp])
                    if g == 0 and kp == KP2 // 2:
                        issue_s2_scalar(1, range(1, KP2 // 2, 2))
                    elif g == 0 and kp == KP2 - 1:
                        issue_s2_scalar(1, range(KP2 // 2 + 1, KP2, 2))
                    for kk in range(2):
                        k = kp * 2 + kk
                        st, sp = (k == 0), (k == KF - 1)
                        for m in range(G2):
                            nc.tensor.matmul(ps_o[m][:],
                                             wt[:, kk, m // 4, (m % 4) * P:(m % 4 + 1) * P],
                                             ht[:, k, :], start=st, stop=sp)
                obuf = opool.tile([P, G2, C], mmdt, tag="o", name=f"ob{g}")
                for m in range(G2):
                    nc.vector.tensor_copy(out=obuf[:, m, :], in_=ps_o[m][:])
                    if m % 2 == 1:
                        nc.scalar.dma_start(out=outT[g, m // 2],
                                            in_=obuf[:, m - 1:m + 1, :])

    nc.compile()
    return nc


def _route(x2d, gate_w, top_k):
    """Replicates the reference gate on host: returns (sel [T,k], cw [T,k])."""
    logits = x2d @ gate_w                       # [T, E] fp32
    sel = np.argsort(-logits, axis=-1, kind="stable")[:, :top_k]
    vals = np.take_along_axis(logits, sel, axis=-1)
    m = vals.max(axis=-1, keepdims=True)
    ex = np.exp(vals - m)
    cw = ex / ex.sum(axis=-1, keepdims=True)
    return sel, cw


def kernel(x, gate_w, w1, w3, w2, top_k):
    from concourse.bass_utils import run_bass_kernel_spmd

    x = np.asarray(x, np.float32)
    gate_w = np.asarray(gate_w, np.float32)
    w1 = np.asarray(w1, np.float32)
    w3 = np.asarray(w3, np.float32)
    w2 = np.asarray(w2, np.float32)
    k = int(top_k)

    x2d = x.reshape(T, D)
    sel, cw = _route(x2d, gate_w, k)

    # token lists per expert
    idx = [np.where((sel == e).any(axis=1))[0] for e in range(E)]
    wgt = []
    for e in range(E):
        m = sel[idx[e]] == e
        wgt.append(cw[idx[e]][m].astype(np.float32))
    counts = np.array([len(i) for i in idx])
    maxc = int(counts.max())
    C = max(160, -(-maxc // 32) * 32)
    n_chunks = 1
    if C > 512:  # capacity overflow: run multiple passes of 512
        C = 512
        n_chunks = -(-maxc // C)

    if C not in _cache:
        _cache[C] = _build(C, w_bufs=24 if C <= 256 else 14)
    nc = _cache[C]

    ndt = _np_bf16()
    wpacked = []
    for e in range(E):
        # w13 [NG1, KD//2, P, kk, w, G1*P]: line = one 4KB block per partition
        w1r = w1[e].astype(ndt).reshape(KP1, 2, P, NG1, G1 * P)
        w3r = w3[e].astype(ndt).reshape(KP1, 2, P, NG1, G1 * P)
        w13 = np.ascontiguousarray(
            np.stack([w1r, w3r], axis=4).transpose(3, 0, 2, 1, 4, 5))
        # w2p [NG2, KF//2, P, kk, 2, G1*P] (same 6D block shape as w13)
        w2r = w2[e].astype(ndt).reshape(KP2, 2, P, NG2, G2 * P)
        w2pk = np.ascontiguousarray(w2r.transpose(3, 0, 2, 1, 4)).reshape(
            NG2, KP2, P, 2, 2, G1 * P)
        wpacked.append((w13, w2pk))

    out = np.zeros((T, D), np.float32)
    for chunk in range(n_chunks):
        in_maps = []
        for e in range(E):
            ide = idx[e][chunk * C:(chunk + 1) * C]
            xTe = np.zeros((D, C), ndt)
            xTe[:, :len(ide)] = x2d[ide].T.astype(ndt)
            in_maps.append({
                "xT": np.ascontiguousarray(
                    xTe.reshape(KD, P, C).transpose(1, 0, 2)),
                "w13": wpacked[e][0],
                "w2p": wpacked[e][1],
            })
        res = run_bass_kernel_spmd(nc, in_maps, core_ids=list(range(E)))
        global last_results
        last_results = res
        for e in range(E):
            ide = idx[e][chunk * C:(chunk + 1) * C]
            if len(ide) == 0:
                continue
            we = wgt[e][chunk * C:(chunk + 1) * C]
            # outT [NG2, 4, P, 2, C] -> [D, C], d = g*1024 + (q*2+mm)*128 + p
            oTe = res.results[e]["outT"].astype(np.float32).transpose(
                0, 1, 3, 2, 4).reshape(D, C)
            # token indices are unique within one expert's list
            out[ide] += we[:, None] * oTe[:, :len(ide)].T

    return out.reshape(B, S, D)


# revision 7
# speedup vs baseline: 1.0878x; 1.0503x over previous
"""MoE top-2 routing kernel for 8 Trainium2 NeuronCores.

Problem (hardcoded shapes): x [64,8,2048] f32, gate_w [2048,8] f32,
w1/w3 [8,2048,4096] f32, w2 [8,4096,2048] f32, top_k=2.

Strategy (expert parallelism):
  - Host computes the gate (512x8 logits, top-2, softmax) exactly as the
    reference does -- ~17 MFLOP, negligible.
  - Tokens are dispatched per expert (gathered + padded to capacity C),
    one expert per NeuronCore.  Each core runs the SwiGLU FFN for its
    expert over its C token slots:
        outT = w2^T @ (silu(w1^T @ xT) * (w3^T @ xT))
    with all matmuls laid out [K, M]/[K, N] so no on-device transposes
    are needed (tokens are the moving free dim).
  - The combine weights are folded into the host-side scatter-add of the
    per-expert outputs back into the [512, 2048] output.

The kernel is DMA-bound: 50.3 MB of bf16 weights per core at the
~358 GB/s HBM-per-NC limit is ~141 us.  Design choices follow:
  - Weight/x loads alternate between sync (SP HWDGE) and gpsimd
    (SWDGE) so two descriptor streams feed the 16 SDMA rings, and no
    compute instruction ever queues behind a blocking dma_start wait
    (v1 lost ~20 us to Silu stalled behind weight-DMA waits on scalar).
  - Weight dma_starts move 512 KB each ([128 partitions x 4KB lines]);
    8KB lines measured ~12%% slower per byte (packet-split overhead).
  - scalar (Activation): act-table prime, 32 Silus, stage-2 out DMAs
    (issued only after all Silus in program order).
  - vector (DVE): memset, 32 tensor_tensor mults, 16 out copies
    (PSUM f32 -> SBUF bf16).
  - Outputs stream out in [128, 2C] chunks right behind the PSUM
    copies so the final DMA after the last matmul is tiny.
"""

import numpy as np

B, S, D, F, E = 64, 8, 2048, 4096, 8
T = B * S  # 512 tokens
P = 128
KD = D // P   # 16 k-tiles, D contraction
KF = F // P   # 32 k-tiles, F contraction
MF = F // P   # 32 m-tiles, stage 1
MD = D // P   # 16 m-tiles, stage 2
G1 = 4        # stage-1 m-tiles per group (4 gate + 4 up PSUM tiles = 8 banks)
G2 = 8        # stage-2 m-tiles per group (8 PSUM tiles = 8 banks)
NG1 = MF // G1          # 8 stage-1 groups (512 cols each)
NG2 = MD // G2          # 2 stage-2 groups (1024 cols each)
KP1 = KD // 2           # 8 weight DMAs per stage-1 group (512 KB each)
KP2 = KF // 2           # 16 weight DMAs per stage-2 group (512 KB each)
WARMUP = 16

_cache = {}
last_results = None  # BassKernelResults of the most recent device run


def _np_bf16():
    import ml_dtypes
    return np.dtype(ml_dtypes.bfloat16)


def _build(C, w_bufs=24):
    import concourse.mybir as mybir
    import concourse.tile as tile
    from concourse import bacc

    nc = bacc.Bacc(None, target_bir_lowering=False)
    f32 = mybir.dt.float32
    mmdt = mybir.dt.bfloat16

    # weights packed on host so each dma_start moves one [128, 4KB] block:
    #   w13 [NG1, KD//2, 128, kk=2, w=2, G1*128]   (w = gate/up select)
    #   w2p [NG2, KF//2, 128, kk=2, 2, G1*128]     (same 6D block shape)
    w13 = nc.declare_dram_parameter("w13", [NG1, KP1, P, 2, 2, G1 * P],
                                    mmdt, isOutput=False)
    w2p = nc.declare_dram_parameter("w2p", [NG2, KP2, P, 2, 2, G1 * P],
                                    mmdt, isOutput=False)
    xT = nc.declare_dram_parameter("xT", [P, KD, C], mmdt, isOutput=False)
    # out chunks: [g, q, p, mm, c] with d = g*1024 + (q*2+mm)*128 + p
    outT = nc.declare_dram_parameter("outT", [NG2, G2 // 2, P, 2, C], mmdt,
                                     isOutput=True)

    with tile.TileContext(nc) as tc:
        with (
            tc.tile_pool(name="xpool", bufs=1) as xpool,
            tc.tile_pool(name="hpool", bufs=1) as hpool,
            tc.tile_pool(name="wpool", bufs=w_bufs) as wpool,
            tc.tile_pool(name="wpoolB", bufs=12) as wpoolB,
            tc.tile_pool(name="psum", bufs=8, space="PSUM") as psum,
            tc.tile_pool(name="spool", bufs=4) as spool,
            tc.tile_pool(name="opool", bufs=2) as opool,
        ):
            xt = xpool.tile([P, KD, C], mmdt)
            ht = hpool.tile([P, KF, C], mmdt)

            # Prime the scalar act table (2.6us of ACT_TABLE_LOAD) during
            # warmup instead of stalling the first real Silu.
            warm = xpool.tile([P, 256], mmdt, name="warm")
            nc.vector.memset(warm[:], 0.0)
            prime = xpool.tile([P, 8], f32, name="prime")
            nc.scalar.activation(prime[:], warm[:, 0:8],
                                 mybir.ActivationFunctionType.Silu)

            # Weight tiles alternate per-kp between sync (deep wpool,
            # carries the backpressure) and scalar (small wpoolB whose
            # reuse distance is ~3 groups, so the buffer-free wait is
            # statically satisfied and the Silus behind those dma_starts
            # in the scalar FIFO never stall -- the v1 priority
            # inversion).  Per-tile alternation keeps ring delivery in
            # consumption order.
            wt0 = wpool.tile([P, 2, 2, G1 * P], mmdt, tag="w", name="wt_0_0")
            nc.sync.dma_start(out=wt0[:], in_=w13[0, 0])
            nc.sync.dma_start(out=xt[:, 0:4, :], in_=xT[:, 0:4, :])
            nc.sync.dma_start(out=xt[:, 4:, :], in_=xT[:, 4:, :])

            # HAM warmup: PE activity covering the cold-clock window while
            # the first weight/x transfers land.
            ps_w = psum.tile([P, C], f32, tag="ps", name="ps_warm")
            for i in range(WARMUP):
                nc.tensor.matmul(ps_w[:], warm[:, :P], warm[:, :C],
                                 start=True, stop=True)

            # stage 1: hT[f, t] = silu(w1^T xT) * (w3^T xT), F-major groups
            for g in range(NG1):
                ps_g = [psum.tile([P, C], f32, tag="ps", name=f"ps_g{g}_{m}")
                        for m in range(G1)]
                ps_u = [psum.tile([P, C], f32, tag="ps", name=f"ps_u{g}_{m}")
                        for m in range(G1)]
                for kp in range(KP1):
                    if kp % 2 == 1:
                        wt = wpoolB.tile([P, 2, 2, G1 * P], mmdt, tag="wB")
                        nc.scalar.dma_start(out=wt[:], in_=w13[g, kp])
                    elif g == 0 and kp == 0:
                        wt = wt0
                    else:
                        wt = wpool.tile([P, 2, 2, G1 * P], mmdt, tag="w")
                        nc.sync.dma_start(out=wt[:], in_=w13[g, kp])
                    for kk in range(2):
                        k = kp * 2 + kk
                        st, sp = (k == 0), (k == KD - 1)
                        for m in range(G1):
                            nc.tensor.matmul(ps_g[m][:], wt[:, kk, 0, m * P:(m + 1) * P],
                                             xt[:, k, :], start=st, stop=sp)
                            nc.tensor.matmul(ps_u[m][:], wt[:, kk, 1, m * P:(m + 1) * P],
                                             xt[:, k, :], start=st, stop=sp)
                for m in range(G1):
                    sig = spool.tile([P, C], f32, tag="sig")
                    nc.scalar.activation(sig[:], ps_g[m][:],
                                         mybir.ActivationFunctionType.Silu)
                    nc.vector.tensor_tensor(out=ht[:, g * G1 + m, :], in0=sig[:],
                                            in1=ps_u[m][:], op=mybir.AluOpType.mult)

            # stage 2: outT[d, t] = w2^T @ hT
            for g in range(NG2):
                ps_o = [psum.tile([P, C], f32, tag="ps", name=f"ps_o{g}_{m}")
                        for m in range(G2)]
                for kp in range(KP2):
                    if kp % 2 == 1:
                        wt = wpoolB.tile([P, 2, 2, G1 * P], mmdt, tag="wB")
                        nc.scalar.dma_start(out=wt[:], in_=w2p[g, kp])
                    else:
                        wt = wpool.tile([P, 2, 2, G1 * P], mmdt, tag="w")
                        nc.sync.dma_start(out=wt[:], in_=w2p[g, kp])
                    for kk in range(2):
                        k = kp * 2 + kk
                        st, sp = (k == 0), (k == KF - 1)
                        for m in range(G2):
                            nc.tensor.matmul(ps_o[m][:],
                                             wt[:, kk, m // 4, (m % 4) * P:(m % 4 + 1) * P],
                                             ht[:, k, :], start=st, stop=sp)
                obuf = opool.tile([P, G2, C], mmdt, tag="o", name=f"ob{g}")
                for m in range(G2):
                    nc.vector.tensor_copy(out=obuf[:, m, :], in_=ps_o[m][:])
                    if m % 2 == 1:
                        nc.scalar.dma_start(out=outT[g, m // 2],
                                            in_=obuf[:, m - 1:m + 1, :])

    nc.compile()
    return nc


def _route(x2d, gate_w, top_k):
    """Replicates the reference gate on host: returns (sel [T,k], cw [T,k])."""
    logits = x2d @ gate_w                       # [T, E] fp32
    sel = np.argsort(-logits, axis=-1, kind="stable")[:, :top_k]
    vals = np.take_along_axis(logits, sel, axis=-1)
    m = vals.max(axis=-1, keepdims=True)
    ex = np.exp(vals - m)
    cw = ex / ex.sum(axis=-1, keepdims=True)
    return sel, cw


def kernel(x, gate_w, w1, w3, w2, top_k):
    from concourse.bass_utils import run_bass_kernel_spmd

    x = np.asarray(x, np.float32)
    gate_w = np.asarray(gate_w, np.float32)
    w1 = np.asarray(w1, np.float32)
    w3 = np.asarray(w3, np.float32)
    w2 = np.asarray(w2, np.float32)
    k = int(top_k)

    x2d = x.reshape(T, D)
    sel, cw = _route(x2d, gate_w, k)

    # token lists per expert
    idx = [np.where((sel == e).any(axis=1))[0] for e in range(E)]
    wgt = []
    for e in range(E):
        m = sel[idx[e]] == e
        wgt.append(cw[idx[e]][m].astype(np.float32))
    counts = np.array([len(i) for i in idx])
    maxc = int(counts.max())
    C = max(160, -(-maxc // 32) * 32)
    n_chunks = 1
    if C > 512:  # capacity overflow: run multiple passes of 512
        C = 512
        n_chunks = -(-maxc // C)

    if C not in _cache:
        _cache[C] = _build(C, w_bufs=24 if C <= 256 else 14)
    nc = _cache[C]

    ndt = _np_bf16()
    wpacked = []
    for e in range(E):
        # w13 [NG1, KD//2, P, kk, w, G1*P]: line = one 4KB block per partition
        w1r = w1[e].astype(ndt).reshape(KP1, 2, P, NG1, G1 * P)
        w3r = w3[e].astype(ndt).reshape(KP1, 2, P, NG1, G1 * P)
        w13 = np.ascontiguousarray(
            np.stack([w1r, w3r], axis=4).transpose(3, 0, 2, 1, 4, 5))
        # w2p [NG2, KF//2, P, kk, 2, G1*P] (same 6D block shape as w13)
        w2r = w2[e].astype(ndt).reshape(KP2, 2, P, NG2, G2 * P)
        w2pk = np.ascontiguousarray(w2r.transpose(3, 0, 2, 1, 4)).reshape(
            NG2, KP2, P, 2, 2, G1 * P)
        wpacked.append((w13, w2pk))

    out = np.zeros((T, D), np.float32)
    for chunk in range(n_chunks):
        in_maps = []
        for e in range(E):
            ide = idx[e][chunk * C:(chunk + 1) * C]
            xTe = np.zeros((D, C), ndt)
            xTe[:, :len(ide)] = x2d[ide].T.astype(ndt)
            in_maps.append({
                "xT": np.ascontiguousarray(
                    xTe.reshape(KD, P, C).transpose(1, 0, 2)),
                "w13": wpacked[e][0],
                "w2p": wpacked[e][1],
            })
        res = run_bass_kernel_spmd(nc, in_maps, core_ids=list(range(E)))
        global last_results
        last_results = res
        for e in range(E):
            ide = idx[e][chunk * C:(chunk + 1) * C]
            if len(ide) == 0:
                continue
            we = wgt[e][chunk * C:(chunk + 1) * C]
            # outT [NG2, 4, P, 2, C] -> [D, C], d = g*1024 + (q*2+mm)*128 + p
            oTe = res.results[e]["outT"].astype(np.float32).transpose(
                0, 1, 3, 2, 4).reshape(D, C)
            # token indices are unique within one expert's list
            out[ide] += we[:, None] * oTe[:, :len(ide)].T

    return out.reshape(B, S, D)


# revision 8
# speedup vs baseline: 1.1106x; 1.0209x over previous
"""MoE top-2 routing kernel for 8 Trainium2 NeuronCores.

Problem (hardcoded shapes): x [64,8,2048] f32, gate_w [2048,8] f32,
w1/w3 [8,2048,4096] f32, w2 [8,4096,2048] f32, top_k=2.

Strategy (expert parallelism):
  - Host computes the gate (512x8 logits, top-2, softmax) exactly as the
    reference does -- ~17 MFLOP, negligible.
  - Tokens are dispatched per expert (gathered + padded to capacity C),
    one expert per NeuronCore.  Each core runs the SwiGLU FFN for its
    expert over its C token slots:
        outT = w2^T @ (silu(w1^T @ xT) * (w3^T @ xT))
    with all matmuls laid out [K, M]/[K, N] so no on-device transposes
    are needed (tokens are the moving free dim).
  - The combine weights are folded into the host-side scatter-add of the
    per-expert outputs back into the [512, 2048] output.

The kernel is DMA-bound: 50.3 MB of bf16 weights per core at the
~358 GB/s HBM-per-NC limit is ~141 us.  Design choices follow:
  - ONE issuing queue (sync/SP HWDGE) carries all weight + x loads so
    no compute instruction ever queues behind a blocking dma_start wait
    (v1 lost ~20 us to Silu stalled behind weight-DMA waits on scalar;
    dual-queue variants measured 6-18 us slower).
  - Weight dma_starts move 512 KB each ([128 partitions x 4KB lines]);
    8KB lines measured ~12%% slower per byte (packet-split overhead).
  - scalar (Activation): act-table prime, 32 Silus, stage-2 out DMAs
    (issued only after all Silus in program order).
  - vector (DVE): memset, 32 tensor_tensor mults, 16 out copies
    (PSUM f32 -> SBUF bf16).
  - Outputs stream out in [128, 2C] chunks right behind the PSUM
    copies so the final DMA after the last matmul is tiny.
"""

import numpy as np

B, S, D, F, E = 64, 8, 2048, 4096, 8
T = B * S  # 512 tokens
P = 128
KD = D // P   # 16 k-tiles, D contraction
KF = F // P   # 32 k-tiles, F contraction
MF = F // P   # 32 m-tiles, stage 1
MD = D // P   # 16 m-tiles, stage 2
G1 = 4        # stage-1 m-tiles per group (4 gate + 4 up PSUM tiles = 8 banks)
G2 = 8        # stage-2 m-tiles per group (8 PSUM tiles = 8 banks)
NG1 = MF // G1          # 8 stage-1 groups (512 cols each)
NG2 = MD // G2          # 2 stage-2 groups (1024 cols each)
KP1 = KD // 2           # 8 weight DMAs per stage-1 group (512 KB each)
KP2 = KF // 2           # 16 weight DMAs per stage-2 group (512 KB each)
WARMUP = 16

_cache = {}
last_results = None  # BassKernelResults of the most recent device run


def _np_bf16():
    import ml_dtypes
    return np.dtype(ml_dtypes.bfloat16)


def _build(C, w_bufs=36):
    import concourse.mybir as mybir
    import concourse.tile as tile
    from concourse import bacc

    nc = bacc.Bacc(None, target_bir_lowering=False)
    f32 = mybir.dt.float32
    mmdt = mybir.dt.bfloat16

    # weights packed on host so each dma_start moves one [128, 4KB] block:
    #   w13 [NG1, KD//2, 128, kk=2, w=2, G1*128]   (w = gate/up select)
    #   w2p [NG2, KF//2, 128, kk=2, 2, G1*128]     (same 6D block shape)
    w13 = nc.declare_dram_parameter("w13", [NG1, KP1, P, 2, 2, G1 * P],
                                    mmdt, isOutput=False)
    w2p = nc.declare_dram_parameter("w2p", [NG2, KP2, P, 2, 2, G1 * P],
                                    mmdt, isOutput=False)
    xT = nc.declare_dram_parameter("xT", [P, KD, C], mmdt, isOutput=False)
    # out chunks: [g, q, p, mm, c] with d = g*1024 + (q*2+mm)*128 + p
    outT = nc.declare_dram_parameter("outT", [NG2, G2 // 2, P, 2, C], mmdt,
                                     isOutput=True)

    with tile.TileContext(nc) as tc:
        with (
            tc.tile_pool(name="xpool", bufs=1) as xpool,
            tc.tile_pool(name="hpool", bufs=1) as hpool,
            tc.tile_pool(name="wpool", bufs=w_bufs) as wpool,
            tc.tile_pool(name="psum", bufs=8, space="PSUM") as psum,
            tc.tile_pool(name="spool", bufs=4) as spool,
            tc.tile_pool(name="opool", bufs=2) as opool,
        ):
            xt = xpool.tile([P, KD, C], mmdt)
            ht = hpool.tile([P, KF, C], mmdt)

            # Prime the scalar act table (2.6us of ACT_TABLE_LOAD) during
            # warmup instead of stalling the first real Silu.
            warm = xpool.tile([P, 256], mmdt, name="warm")
            nc.vector.memset(warm[:], 0.0)
            prime = xpool.tile([P, 8], f32, name="prime")
            nc.scalar.activation(prime[:], warm[:, 0:8],
                                 mybir.ActivationFunctionType.Silu)

            # First weight tile, then x (the first matmul needs both);
            # remaining weight issues follow in consumption order.
            wt0 = wpool.tile([P, 2, 2, G1 * P], mmdt, tag="w", name="wt_0_0")
            nc.sync.dma_start(out=wt0[:], in_=w13[0, 0])
            nc.sync.dma_start(out=xt[:, 0:4, :], in_=xT[:, 0:4, :])
            nc.sync.dma_start(out=xt[:, 4:, :], in_=xT[:, 4:, :])

            # HAM warmup: PE activity covering the cold-clock window while
            # the first weight/x transfers land.
            ps_w = psum.tile([P, C], f32, tag="ps", name="ps_warm")
            for i in range(WARMUP):
                nc.tensor.matmul(ps_w[:], warm[:, :P], warm[:, :C],
                                 start=True, stop=True)

            # stage 1: hT[f, t] = silu(w1^T xT) * (w3^T xT), F-major groups
            for g in range(NG1):
                ps_g = [psum.tile([P, C], f32, tag="ps", name=f"ps_g{g}_{m}")
                        for m in range(G1)]
                ps_u = [psum.tile([P, C], f32, tag="ps", name=f"ps_u{g}_{m}")
                        for m in range(G1)]
                for kp in range(KP1):
                    if g == 0 and kp == 0:
                        wt = wt0
                    else:
                        wt = wpool.tile([P, 2, 2, G1 * P], mmdt, tag="w")
                        nc.sync.dma_start(out=wt[:], in_=w13[g, kp])
                    for kk in range(2):
                        k = kp * 2 + kk
                        st, sp = (k == 0), (k == KD - 1)
                        for m in range(G1):
                            nc.tensor.matmul(ps_g[m][:], wt[:, kk, 0, m * P:(m + 1) * P],
                                             xt[:, k, :], start=st, stop=sp)
                            nc.tensor.matmul(ps_u[m][:], wt[:, kk, 1, m * P:(m + 1) * P],
                                             xt[:, k, :], start=st, stop=sp)
                for m in range(G1):
                    sig = spool.tile([P, C], f32, tag="sig")
                    nc.scalar.activation(sig[:], ps_g[m][:],
                                         mybir.ActivationFunctionType.Silu)
                    nc.vector.tensor_tensor(out=ht[:, g * G1 + m, :], in0=sig[:],
                                            in1=ps_u[m][:], op=mybir.AluOpType.mult)

            # stage 2: outT[d, t] = w2^T @ hT
            for g in range(NG2):
                ps_o = [psum.tile([P, C], f32, tag="ps", name=f"ps_o{g}_{m}")
                        for m in range(G2)]
                for kp in range(KP2):
                    wt = wpool.tile([P, 2, 2, G1 * P], mmdt, tag="w")
                    nc.sync.dma_start(out=wt[:], in_=w2p[g, kp])
                    for kk in range(2):
                        k = kp * 2 + kk
                        st, sp = (k == 0), (k == KF - 1)
                        for m in range(G2):
                            nc.tensor.matmul(ps_o[m][:],
                                             wt[:, kk, m // 4, (m % 4) * P:(m % 4 + 1) * P],
                                             ht[:, k, :], start=st, stop=sp)
                obuf = opool.tile([P, G2, C], mmdt, tag="o", name=f"ob{g}")
                for m in range(G2):
                    nc.vector.tensor_copy(out=obuf[:, m, :], in_=ps_o[m][:])
                    if m % 2 == 1:
                        nc.scalar.dma_start(out=outT[g, m // 2],
                                            in_=obuf[:, m - 1:m + 1, :])

    nc.compile()
    return nc


def _route(x2d, gate_w, top_k):
    """Replicates the reference gate on host: returns (sel [T,k], cw [T,k])."""
    logits = x2d @ gate_w                       # [T, E] fp32
    sel = np.argsort(-logits, axis=-1, kind="stable")[:, :top_k]
    vals = np.take_along_axis(logits, sel, axis=-1)
    m = vals.max(axis=-1, keepdims=True)
    ex = np.exp(vals - m)
    cw = ex / ex.sum(axis=-1, keepdims=True)
    return sel, cw


def kernel(x, gate_w, w1, w3, w2, top_k):
    from concourse.bass_utils import run_bass_kernel_spmd

    x = np.asarray(x, np.float32)
    gate_w = np.asarray(gate_w, np.float32)
    w1 = np.asarray(w1, np.float32)
    w3 = np.asarray(w3, np.float32)
    w2 = np.asarray(w2, np.float32)
    k = int(top_k)

    x2d = x.reshape(T, D)
    sel, cw = _route(x2d, gate_w, k)

    # token lists per expert
    idx = [np.where((sel == e).any(axis=1))[0] for e in range(E)]
    wgt = []
    for e in range(E):
        m = sel[idx[e]] == e
        wgt.append(cw[idx[e]][m].astype(np.float32))
    counts = np.array([len(i) for i in idx])
    maxc = int(counts.max())
    C = max(160, -(-maxc // 32) * 32)
    n_chunks = 1
    if C > 512:  # capacity overflow: run multiple passes of 512
        C = 512
        n_chunks = -(-maxc // C)

    if C not in _cache:
        _cache[C] = _build(C, w_bufs=36 if C <= 256 else 20)
    nc = _cache[C]

    ndt = _np_bf16()
    wpacked = []
    for e in range(E):
        # w13 [NG1, KD//2, P, kk, w, G1*P]: line = one 4KB block per partition
        w1r = w1[e].astype(ndt).reshape(KP1, 2, P, NG1, G1 * P)
        w3r = w3[e].astype(ndt).reshape(KP1, 2, P, NG1, G1 * P)
        w13 = np.ascontiguousarray(
            np.stack([w1r, w3r], axis=4).transpose(3, 0, 2, 1, 4, 5))
        # w2p [NG2, KF//2, P, kk, 2, G1*P] (same 6D block shape as w13)
        w2r = w2[e].astype(ndt).reshape(KP2, 2, P, NG2, G2 * P)
        w2pk = np.ascontiguousarray(w2r.transpose(3, 0, 2, 1, 4)).reshape(
            NG2, KP2, P, 2, 2, G1 * P)
        wpacked.append((w13, w2pk))

    out = np.zeros((T, D), np.float32)
    for chunk in range(n_chunks):
        in_maps = []
        for e in range(E):
            ide = idx[e][chunk * C:(chunk + 1) * C]
            xTe = np.zeros((D, C), ndt)
            xTe[:, :len(ide)] = x2d[ide].T.astype(ndt)
            in_maps.append({
                "xT": np.ascontiguousarray(
                    xTe.reshape(KD, P, C).transpose(1, 0, 2)),
                "w13": wpacked[e][0],
                "w2p": wpacked[e][1],
            })
        res = run_bass_kernel_spmd(nc, in_maps, core_ids=list(range(E)))
        global last_results
        last_results = res
        for e in range(E):
            ide = idx[e][chunk * C:(chunk + 1) * C]
            if len(ide) == 0:
                continue
            we = wgt[e][chunk * C:(chunk + 1) * C]
            # outT [NG2, 4, P, 2, C] -> [D, C], d = g*1024 + (q*2+mm)*128 + p
            oTe = res.results[e]["outT"].astype(np.float32).transpose(
                0, 1, 3, 2, 4).reshape(D, C)
            # token indices are unique within one expert's list
            out[ide] += we[:, None] * oTe[:, :len(ide)].T

    return out.reshape(B, S, D)


# revision 9
# speedup vs baseline: 1.1990x; 1.0797x over previous
"""MoE top-2 routing kernel for 8 Trainium2 NeuronCores.

Problem (hardcoded shapes): x [64,8,2048] f32, gate_w [2048,8] f32,
w1/w3 [8,2048,4096] f32, w2 [8,4096,2048] f32, top_k=2.

Strategy (expert parallelism):
  - Host computes the gate (512x8 logits, top-2, softmax) exactly as the
    reference does -- ~17 MFLOP, negligible.
  - Tokens are dispatched per expert (gathered + padded to capacity C),
    one expert per NeuronCore.  Each core runs the SwiGLU FFN for its
    expert over its C token slots:
        outT = w2^T @ (silu(w1^T @ xT) * (w3^T @ xT))
    with all matmuls laid out [K, M]/[K, N] so no on-device transposes
    are needed (tokens are the moving free dim).
  - The combine weights are folded into the host-side scatter-add of the
    per-expert outputs back into the [512, 2048] output.

The kernel is DMA-bound: 50.3 MB of bf16 weights per core at the
~358 GB/s HBM-per-NC limit is ~141 us.  Design choices follow:
  - ONE issuing queue (sync/SP HWDGE) carries all weight + x loads so
    no compute instruction ever queues behind a blocking dma_start wait
    (v1 lost ~20 us to Silu stalled behind weight-DMA waits on scalar;
    dual-queue variants measured 6-18 us slower).
  - Weight dma_starts move 1 MB each but split into 4KB descriptors
    (max_dma_last_dim=2048): 8KB descriptors measured ~12% slower per
    byte, while fewer dma_starts halve the semaphore-inc descriptors.
  - scalar (Activation): act-table prime, 32 Silus, stage-2 out DMAs
    (issued only after all Silus in program order).
  - vector (DVE): memset, 32 tensor_tensor mults, 16 out copies
    (PSUM f32 -> SBUF bf16).
  - Outputs stream out in [128, 2C] chunks right behind the PSUM
    copies so the final DMA after the last matmul is tiny.
"""

import numpy as np

B, S, D, F, E = 64, 8, 2048, 4096, 8
T = B * S  # 512 tokens
P = 128
KD = D // P   # 16 k-tiles, D contraction
KF = F // P   # 32 k-tiles, F contraction
MF = F // P   # 32 m-tiles, stage 1
MD = D // P   # 16 m-tiles, stage 2
G1 = 4        # stage-1 m-tiles per group (4 gate + 4 up PSUM tiles = 8 banks)
G2 = 8        # stage-2 m-tiles per group (8 PSUM tiles = 8 banks)
NG1 = MF // G1          # 8 stage-1 groups (512 cols each)
NG2 = MD // G2          # 2 stage-2 groups (1024 cols each)
KP1 = KD // 4           # 4 weight DMAs per stage-1 group (1 MB each)
KP2 = KF // 4           # 8 weight DMAs per stage-2 group (1 MB each)
WARMUP = 16

_cache = {}
last_results = None  # BassKernelResults of the most recent device run


def _np_bf16():
    import ml_dtypes
    return np.dtype(ml_dtypes.bfloat16)


def _build(C, w_bufs=18):
    import concourse.mybir as mybir
    import concourse.tile as tile
    from concourse import bacc

    nc = bacc.Bacc(None, target_bir_lowering=False)
    f32 = mybir.dt.float32
    mmdt = mybir.dt.bfloat16

    # weights packed on host so each dma_start moves one [128, 4KB] block:
    #   w13 [NG1, KD//2, 128, kk=2, w=2, G1*128]   (w = gate/up select)
    #   w2p [NG2, KF//2, 128, kk=2, 2, G1*128]     (same 6D block shape)
    w13 = nc.declare_dram_parameter("w13", [NG1, KP1, P, 4, 2, G1 * P],
                                    mmdt, isOutput=False)
    w2p = nc.declare_dram_parameter("w2p", [NG2, KP2, P, 4, 2, G1 * P],
                                    mmdt, isOutput=False)
    xT = nc.declare_dram_parameter("xT", [P, KD, C], mmdt, isOutput=False)
    # out chunks: [g, q, p, mm, c] with d = g*1024 + (q*2+mm)*128 + p
    outT = nc.declare_dram_parameter("outT", [NG2, G2 // 2, P, 2, C], mmdt,
                                     isOutput=True)

    with tile.TileContext(nc) as tc:
        with (
            tc.tile_pool(name="xpool", bufs=1) as xpool,
            tc.tile_pool(name="hpool", bufs=1) as hpool,
            tc.tile_pool(name="wpool", bufs=w_bufs) as wpool,
            tc.tile_pool(name="psum", bufs=8, space="PSUM") as psum,
            tc.tile_pool(name="spool", bufs=4) as spool,
            tc.tile_pool(name="opool", bufs=2) as opool,
        ):
            xt = xpool.tile([P, KD, C], mmdt)
            ht = hpool.tile([P, KF, C], mmdt)

            # Prime the scalar act table (2.6us of ACT_TABLE_LOAD) during
            # warmup instead of stalling the first real Silu.
            warm = xpool.tile([P, 256], mmdt, name="warm")
            nc.vector.memset(warm[:], 0.0)
            prime = xpool.tile([P, 8], f32, name="prime")
            nc.scalar.activation(prime[:], warm[:, 0:8],
                                 mybir.ActivationFunctionType.Silu)

            # First weight tile, then x (the first matmul needs both);
            # remaining weight issues follow in consumption order.
            wt0 = wpool.tile([P, 4, 2, G1 * P], mmdt, tag="w", name="wt_0_0")
            nc.sync.dma_start(out=wt0[:], in_=w13[0, 0], max_dma_last_dim=2048)
            nc.sync.dma_start(out=xt[:, 0:4, :], in_=xT[:, 0:4, :])
            nc.sync.dma_start(out=xt[:, 4:, :], in_=xT[:, 4:, :])

            # HAM warmup: PE activity covering the cold-clock window while
            # the first weight/x transfers land.
            ps_w = psum.tile([P, C], f32, tag="ps", name="ps_warm")
            for i in range(WARMUP):
                nc.tensor.matmul(ps_w[:], warm[:, :P], warm[:, :C],
                                 start=True, stop=True)

            # stage 1: hT[f, t] = silu(w1^T xT) * (w3^T xT), F-major groups
            for g in range(NG1):
                ps_g = [psum.tile([P, C], f32, tag="ps", name=f"ps_g{g}_{m}")
                        for m in range(G1)]
                ps_u = [psum.tile([P, C], f32, tag="ps", name=f"ps_u{g}_{m}")
                        for m in range(G1)]
                for kp in range(KP1):
                    if g == 0 and kp == 0:
                        wt = wt0
                    else:
                        wt = wpool.tile([P, 4, 2, G1 * P], mmdt, tag="w")
                        nc.sync.dma_start(out=wt[:], in_=w13[g, kp],
                                          max_dma_last_dim=2048)
                    for kk in range(4):
                        k = kp * 4 + kk
                        st, sp = (k == 0), (k == KD - 1)
                        for m in range(G1):
                            nc.tensor.matmul(ps_g[m][:], wt[:, kk, 0, m * P:(m + 1) * P],
                                             xt[:, k, :], start=st, stop=sp)
                            nc.tensor.matmul(ps_u[m][:], wt[:, kk, 1, m * P:(m + 1) * P],
                                             xt[:, k, :], start=st, stop=sp)
                for m in range(G1):
                    sig = spool.tile([P, C], f32, tag="sig")
                    nc.scalar.activation(sig[:], ps_g[m][:],
                                         mybir.ActivationFunctionType.Silu)
                    nc.vector.tensor_tensor(out=ht[:, g * G1 + m, :], in0=sig[:],
                                            in1=ps_u[m][:], op=mybir.AluOpType.mult)

            # stage 2: outT[d, t] = w2^T @ hT
            for g in range(NG2):
                ps_o = [psum.tile([P, C], f32, tag="ps", name=f"ps_o{g}_{m}")
                        for m in range(G2)]
                for kp in range(KP2):
                    wt = wpool.tile([P, 4, 2, G1 * P], mmdt, tag="w")
                    nc.sync.dma_start(out=wt[:], in_=w2p[g, kp],
                                      max_dma_last_dim=2048)
                    for kk in range(4):
                        k = kp * 4 + kk
                        st, sp = (k == 0), (k == KF - 1)
                        for m in range(G2):
                            nc.tensor.matmul(ps_o[m][:],
                                             wt[:, kk, m // 4, (m % 4) * P:(m % 4 + 1) * P],
                                             ht[:, k, :], start=st, stop=sp)
                obuf = opool.tile([P, G2, C], mmdt, tag="o", name=f"ob{g}")
                for m in range(G2):
                    nc.vector.tensor_copy(out=obuf[:, m, :], in_=ps_o[m][:])
                    if m % 2 == 1:
                        nc.scalar.dma_start(out=outT[g, m // 2],
                                            in_=obuf[:, m - 1:m + 1, :])

    nc.compile()
    return nc


def _route(x2d, gate_w, top_k):
    """Replicates the reference gate on host: returns (sel [T,k], cw [T,k])."""
    logits = x2d @ gate_w                       # [T, E] fp32
    sel = np.argsort(-logits, axis=-1, kind="stable")[:, :top_k]
    vals = np.take_along_axis(logits, sel, axis=-1)
    m = vals.max(axis=-1, keepdims=True)
    ex = np.exp(vals - m)
    cw = ex / ex.sum(axis=-1, keepdims=True)
    return sel, cw


def kernel(x, gate_w, w1, w3, w2, top_k):
    from concourse.bass_utils import run_bass_kernel_spmd

    x = np.asarray(x, np.float32)
    gate_w = np.asarray(gate_w, np.float32)
    w1 = np.asarray(w1, np.float32)
    w3 = np.asarray(w3, np.float32)
    w2 = np.asarray(w2, np.float32)
    k = int(top_k)

    x2d = x.reshape(T, D)
    sel, cw = _route(x2d, gate_w, k)

    # token lists per expert
    idx = [np.where((sel == e).any(axis=1))[0] for e in range(E)]
    wgt = []
    for e in range(E):
        m = sel[idx[e]] == e
        wgt.append(cw[idx[e]][m].astype(np.float32))
    counts = np.array([len(i) for i in idx])
    maxc = int(counts.max())
    C = max(160, -(-maxc // 32) * 32)
    n_chunks = 1
    if C > 512:  # capacity overflow: run multiple passes of 512
        C = 512
        n_chunks = -(-maxc // C)

    if C not in _cache:
        _cache[C] = _build(C, w_bufs=18 if C <= 256 else 10)
    nc = _cache[C]

    ndt = _np_bf16()
    wpacked = []
    for e in range(E):
        # w13 [NG1, KD//2, P, kk, w, G1*P]: line = one 4KB block per partition
        w1r = w1[e].astype(ndt).reshape(KP1, 4, P, NG1, G1 * P)
        w3r = w3[e].astype(ndt).reshape(KP1, 4, P, NG1, G1 * P)
        w13 = np.ascontiguousarray(
            np.stack([w1r, w3r], axis=4).transpose(3, 0, 2, 1, 4, 5))
        # w2p [NG2, KF//2, P, kk, 2, G1*P] (same 6D block shape as w13)
        w2r = w2[e].astype(ndt).reshape(KP2, 4, P, NG2, G2 * P)
        w2pk = np.ascontiguousarray(w2r.transpose(3, 0, 2, 1, 4)).reshape(
            NG2, KP2, P, 4, 2, G1 * P)
        wpacked.append((w13, w2pk))

    out = np.zeros((T, D), np.float32)
    for chunk in range(n_chunks):
        in_maps = []
        for e in range(E):
            ide = idx[e][chunk * C:(chunk + 1) * C]
            xTe = np.zeros((D, C), ndt)
            xTe[:, :len(ide)] = x2d[ide].T.astype(ndt)
            in_maps.append({
                "xT": np.ascontiguousarray(
                    xTe.reshape(KD, P, C).transpose(1, 0, 2)),
                "w13": wpacked[e][0],
                "w2p": wpacked[e][1],
            })
        res = run_bass_kernel_spmd(nc, in_maps, core_ids=list(range(E)))
        global last_results
        last_results = res
        for e in range(E):
            ide = idx[e][chunk * C:(chunk + 1) * C]
            if len(ide) == 0:
                continue
            we = wgt[e][chunk * C:(chunk + 1) * C]
            # outT [NG2, 4, P, 2, C] -> [D, C], d = g*1024 + (q*2+mm)*128 + p
            oTe = res.results[e]["outT"].astype(np.float32).transpose(
                0, 1, 3, 2, 4).reshape(D, C)
            # token indices are unique within one expert's list
            out[ide] += we[:, None] * oTe[:, :len(ide)].T

    return out.reshape(B, S, D)
